# revision 2
# baseline (speedup 1.0000x reference)
"""Bidirectional ConvLSTM encoder kernel for Trainium2 (Bass/Tile).

Problem: B=8, T=16, C=3, H=W=32, HID=64, 7x7 convs, bidirectional.
Sharding: data-parallel over batch; core b handles batch element b, running
both the forward and backward recurrences (2 independent recurrences that
ping-pong on the PE so gate/elementwise latency of one hides under the
other's matmuls).

Conv formulation: hidden 7x7 conv (64->256ch) as a sum of shifted matmuls
over a zero-padded [64, 38, 38] state image, taps packed in pairs onto the
128-deep contraction dim via two shifted state copies (see pack_whh).
25 matmuls replace 49. The 3x7x7 input conv is im2col'd (K=147, padded to
160) and accumulated into the same PSUM banks.

v3 over baseline:
- K=32 input-conv tile: weights+xcol replicated at partition offsets
  0/32/64/96 so the four (mg,nh) matmuls run CONCURRENTLY in distinct
  32-row groups of the PE via tile_position (4 MMs -> ~1 MM span).
- Singleton hidden tap (6,6) (K=64): weights duplicated on partitions
  64:128; the nh=1 matmul reads the col-shifted upper state copy at column
  base 5 (same values) and runs in row groups 2-3 concurrently with nh=0
  in row groups 0-1 via tile_position (2 MMs -> ~1 MM span per mg).
- Startup: only the t=0 slice of the im2col input is DMA'd before compute
  starts (full tensor was ~4MB, ~12us of PE idle); weights and the
  remaining timesteps stream in behind it.
"""

import numpy as np

HID = 64
T = 16
CIN = 3
H = 32
W = 32
HWSZ = H * W
PW = 38  # padded image width (32 + 2*3)
PAD = 3
KS = 7
NCORES = 8
KIN = CIN * KS * KS  # 147
KIN_PAD = 160  # 128 + 32
# xcol timestep slots in PE consumption order: t=0 fwd, t=15 bwd, t=1 fwd, ...
PERM = [0, 15, 1, 14, 2, 13, 3, 12, 4, 11, 5, 10, 6, 9, 7, 8]
SLOT = {t: PERM.index(t) for t in range(T)}
# whh pair slots in PE consumption order: S pair (index 24) first, then 0..23
PPERM = [24] + list(range(24))
PSLOT = {p: PPERM.index(p) for p in range(25)}

# Hidden-conv tap pairs: (kind, kh, kw).
#  "A": taps (kh, kw) + (kh+1, kw) via the row-shifted upper copy.
#  "B": taps (6, kw) + (6, kw+1) via the col-shifted upper copy.
#  "S": singleton tap (6, 6), K=64, row-tiled 2x (nh=0 lo / nh=1 hi).
PAIRS = (
    [("A", kh0, kw) for kw in range(KS) for kh0 in (0, 2, 4)]
    + [("B", 6, kw0) for kw0 in (0, 2, 4)]
    + [("S", 6, 6)]
)
NPAIR = len(PAIRS)  # 25


def pack_whh(w_hh_f: np.ndarray, w_hh_b: np.ndarray) -> np.ndarray:
    """Pack hidden weights into lhsT tiles: [128(k), 2(dir), 25(pair), 2(mg), 128(m)].

    lhsT[k, d, p, mg, m] so that matmul(lhsT.T @ rhs) with rhs rows
    (k<64: tap_lo channel k, k>=64: tap_hi channel k-64) accumulates the conv.
    The S pair carries the same tap on both halves (row-tiled concurrent MMs).
    """
    out = np.zeros((2, NPAIR, 2, 128, 128), np.float32)  # d, p, mg, k, m
    for d, wsrc in enumerate([w_hh_f, w_hh_b]):
        wsrc = np.asarray(wsrc, dtype=np.float32)  # [256, 64, 7, 7]
        for p, (kind, r, c) in enumerate(PAIRS):
            if kind == "A":
                lo, hi = (r, c), (r + 1, c)
            elif kind == "B":
                lo, hi = (r, c), (r, c + 1)
            else:
                lo, hi = (r, c), (r, c)  # duplicated for the row-tiled variant
            for mg in range(2):
                wm = wsrc[mg * 128 : (mg + 1) * 128]  # [128, 64, 7, 7]
                out[d, p, mg, 0:64, :] = wm[:, :, lo[0], lo[1]].T
                out[d, p, mg, 64:128, :] = wm[:, :, hi[0], hi[1]].T
    out = out[:, PPERM]  # consumption-ordered pair slots
    return np.ascontiguousarray(out.transpose(3, 0, 1, 2, 4).astype(np.float16))  # [k, d, p, mg, m]


def pack_wih(w_ih_f: np.ndarray, w_ih_b: np.ndarray) -> tuple[np.ndarray, np.ndarray]:
    """Input weights (im2col).

    wih0: [128(k), 2(dir), 2(mg), 128(m)] — first K tile.
    wih1: [128(k), 2(dir), 2(mg), 128(m)] — rows 128:160 replicated at
          partition offsets 0/32/64/96 for 4x row-tiled matmuls.
    """
    w0 = np.zeros((128, 2, 2, 128), np.float32)
    w1 = np.zeros((128, 2, 2, 128), np.float32)
    for d, wsrc in enumerate([w_ih_f, w_ih_b]):
        wk = np.asarray(wsrc, dtype=np.float32).reshape(256, KIN)  # (cin,kh,kw) C-order
        for mg in range(2):
            w0[:, d, mg, :] = wk[mg * 128 : (mg + 1) * 128, 0:128].T
            tail = wk[mg * 128 : (mg + 1) * 128, 128:KIN].T  # [19, 128]
            for i in range(4):
                w1[32 * i : 32 * i + 19, d, mg, :] = tail
    return (
        np.ascontiguousarray(w0.astype(np.float16)),
        np.ascontiguousarray(w1.astype(np.float16)),
    )


def pack_bias(b_ih_f, b_hh_f, b_ih_b, b_hh_b) -> np.ndarray:
    """[128(k), 2(dir), 2(mg)]: per-gate-channel bias."""
    out = np.zeros((128, 2, 2), np.float32)
    for d, (bi, bh) in enumerate([(b_ih_f, b_hh_f), (b_ih_b, b_hh_b)]):
        s = np.asarray(bi, dtype=np.float32) + np.asarray(bh, dtype=np.float32)  # [256]
        out[:, d, 0] = s[0:128]
        out[:, d, 1] = s[128:256]
    return np.ascontiguousarray(out)


def pack_xcol(xb: np.ndarray) -> tuple[np.ndarray, np.ndarray]:
    """im2col one batch element [T,3,32,32].

    xcola: [128(k), T, 2, 512] — first K tile.
    xcolb: [128(k), T, 2, 512] — rows 128:147 replicated at partition
           offsets 0/32/64/96 (matching pack_wih's wih1).
    """
    xb = np.asarray(xb, dtype=np.float32)
    xpad = np.pad(xb, ((0, 0), (0, 0), (PAD, PAD), (PAD, PAD)))
    win = np.lib.stride_tricks.sliding_window_view(xpad, (KS, KS), axis=(2, 3))
    # win: [T, 3, 32, 32, 7, 7] -> [(cin, kh, kw), T, hw]
    xcol = win.transpose(1, 4, 5, 0, 2, 3).reshape(KIN, T, HWSZ).astype(np.float16)
    xcol = xcol[:, PERM]  # consumption-ordered slots
    xa = np.ascontiguousarray(xcol[0:128].reshape(128, T, 2, 512))
    tail = xcol[128:KIN].reshape(19, T, 2, 512)
    xb4 = np.zeros((128, T, 2, 512), np.float16)
    for i in range(4):
        xb4[32 * i : 32 * i + 19] = tail
    return xa, xb4


def build_nc():
    import concourse.mybir as mybir
    from concourse import bacc
    from concourse.tile import TileContext

    F32 = mybir.dt.float32
    F16 = mybir.dt.float16
    AF = mybir.ActivationFunctionType

    nc = bacc.Bacc()
    xcola_d = nc.declare_dram_parameter("xcola", [128, T, 2, 512], F16, isOutput=False)
    xcolb_d = nc.declare_dram_parameter("xcolb", [128, T, 2, 512], F16, isOutput=False)
    whh_d = nc.declare_dram_parameter("whh", [128, 2, NPAIR, 2, 128], F16, isOutput=False)
    wih0_d = nc.declare_dram_parameter("wih0", [128, 2, 2, 128], F16, isOutput=False)
    wih1_d = nc.declare_dram_parameter("wih1", [128, 2, 2, 128], F16, isOutput=False)
    bias_d = nc.declare_dram_parameter("bias", [128, 2, 2], F32, isOutput=False)
    out_d = nc.declare_dram_parameter("out", [T, 2, HID, H, W], F32, isOutput=True)

    with TileContext(nc) as tc:
        with (
            tc.tile_pool(name="wpool", bufs=1) as wpool,
            tc.tile_pool(name="state", bufs=1) as spool,
            tc.tile_pool(name="xin", bufs=1) as xpool,
            tc.tile_pool(name="work", bufs=1) as wkpool,
            tc.tile_pool(name="psum", bufs=1, space="PSUM") as pspool,
        ):
            whh = wpool.tile([128, 2, NPAIR, 2, 128], F16)
            wih0 = wpool.tile([128, 2, 2, 128], F16)
            wih1 = wpool.tile([128, 2, 2, 128], F16)
            bias = wpool.tile([128, 2, 2], F32)
            xaF = xpool.tile([128, T, 2, 512], F16)
            xbF = xpool.tile([128, T, 2, 512], F16)

            # Startup choreography. xcol slots are in consumption order
            # (PERM), so chunks are contiguous. sync and scalar are
            # independent FIFO HW-DGE rings: sync carries only the small
            # critical head (so the per-step gate DMAs behind it are never
            # delayed); scalar carries six big bulk transfers whose
            # deadlines are far out. Activations on the scalar engine only
            # have to get past 6 quick DMA issues.
            nc.sync.dma_start(wih0[:], wih0_d[:])
            nc.sync.dma_start(wih1[:], wih1_d[:])
            nc.sync.dma_start(bias[:], bias_d[:])
            nc.sync.dma_start(xaF[:, 0:1, 0], xcola_d[:, 0:1, 0])
            nc.sync.dma_start(xaF[:, 0:1, 1], xcola_d[:, 0:1, 1])
            nc.sync.dma_start(xbF[:, 0:1], xcolb_d[:, 0:1])
            nc.sync.dma_start(xaF[:, 1:2], xcola_d[:, 1:2])
            nc.sync.dma_start(xbF[:, 1:2], xcolb_d[:, 1:2])
            nc.sync.dma_start(xaF[:, 2:4], xcola_d[:, 2:4])
            nc.scalar.dma_start(whh[:, 0, 0:9], whh_d[:, 0, 0:9])
            nc.scalar.dma_start(xbF[:, 2:4], xcolb_d[:, 2:4])
            nc.scalar.dma_start(whh[:, 0, 9:25], whh_d[:, 0, 9:25])

            hAB = [spool.tile([128, 2, PW, PW], F16, tag=f"hAB{d}", name=f"hAB{d}") for d in range(2)]
            # cell state lives on partitions 64-127, where the f and o gates land
            c2 = [spool.tile([128, HWSZ], F32, tag=f"c{d}", name=f"c{d}") for d in range(2)]
            for tl in hAB:
                nc.vector.memset(tl[:], 0.0)

            for t in range(T):
                if t == 1:
                    # bulk transfers deferred so the startup head gets the
                    # full HBM bandwidth; these are needed from ~60us on
                    nc.scalar.dma_start(whh[:, 1, 0:9], whh_d[:, 1, 0:9])
                    nc.scalar.dma_start(whh[:, 1, 9:25], whh_d[:, 1, 9:25])
                    nc.sync.dma_start(xaF[:, 4:10], xcola_d[:, 4:10])
                    nc.scalar.dma_start(xbF[:, 4:10], xcolb_d[:, 4:10])
                elif t == 2:
                    nc.sync.dma_start(xaF[:, 10:16], xcola_d[:, 10:16])
                    nc.scalar.dma_start(xbF[:, 10:16], xcolb_d[:, 10:16])
                for d in range(2):
                    tsrc = t if d == 0 else T - 1 - t

                    ps0 = pspool.tile([128, 2, 512], F32, tag=f"ps{d}0")
                    ps1 = pspool.tile([128, 2, 512], F32, tag=f"ps{d}1")
                    pst = [ps0, ps1]

                    # K=128 input tile: start of each (mg, nh) group
                    for mg in range(2):
                        for nh in range(2):
                            nc.tensor.matmul(
                                pst[mg][:, nh], wih0[:, d, mg], xaF[:, SLOT[tsrc], nh],
                                start=True, stop=False,
                            )
                    # K=32 input tail: all four (mg, nh) matmuls adjacent so
                    # they run concurrently in distinct 32-row groups
                    for mg in range(2):
                        for nh in range(2):
                            i = 2 * mg + nh
                            nc.tensor.matmul(
                                pst[mg][:, nh],
                                wih1[32 * i : 32 * i + 32, d, mg],
                                xbF[32 * i : 32 * i + 32, SLOT[tsrc], nh],
                                start=False, stop=(t == 0),
                                tile_position=(32 * i, 0),
                            )
                    if t > 0:
                        # S singleton (K=64) row-tiled pairs, grouped right
                        # after the K=32 matmuls so the PE pays the tile-
                        # mode-switch stall once per step instead of per
                        # group. nh=0 on rows 0:64 (plain state in B-copy
                        # lower), nh=1 on rows 64:128 (col-shifted B-upper
                        # at column base 5 — same values).
                        ps_idx = PSLOT[NPAIR - 1]
                        for mg in range(2):
                            nc.tensor.matmul(
                                pst[mg][:, 0],
                                whh[0:64, d, ps_idx, mg],
                                hAB[d][0:64, 1, 6:22, 6:38],
                                start=False, stop=False,
                                tile_position=(0, 0),
                            )
                            nc.tensor.matmul(
                                pst[mg][:, 1],
                                whh[64:128, d, ps_idx, mg],
                                hAB[d][64:128, 1, 22:38, 5:37],
                                start=False, stop=False,
                                tile_position=(64, 0),
                            )
                        for mg in range(2):
                            for p, (kind, r, c) in enumerate(PAIRS[:-1]):
                                if kind == "A":
                                    rhf = lambda nh, r=r, c=c: hAB[d][:, 0, r + 16 * nh : r + 16 * nh + 16, c : c + 32]
                                else:
                                    rhf = lambda nh, c=c: hAB[d][:, 1, 6 + 16 * nh : 6 + 16 * nh + 16, c : c + 32]
                                last = p == NPAIR - 2
                                for nh in range(2):
                                    nc.tensor.matmul(
                                        pst[mg][:, nh],
                                        whh[:, d, PSLOT[p], mg],
                                        rhf(nh),
                                        start=False, stop=last,
                                    )

                    sif = wkpool.tile([128, 2, 512], F32, tag=f"sif{d}")
                    sgo = wkpool.tile([128, 2, 512], F32, tag=f"sgo{d}")
                    tmp = wkpool.tile([HID, HWSZ], F32, tag=f"tmp{d}")
                    tup = wkpool.tile([128, HWSZ], F32, tag=f"tup{d}")
                    h2c = wkpool.tile([128, HWSZ], F32, tag=f"h2c{d}{t % 2}")
                    hl = wkpool.tile([HID, HWSZ], F32, tag=f"hl{d}")

                    # gates: i,f = sigmoid(mg0); g = tanh(mg1 lo); o = sigmoid(mg1 hi).
                    # Normally the whole chain hides under the other direction's
                    # matmul stream; at the kernel head (t=0 d=0, stalls t=1's
                    # hidden matmuls) and tail (last step d=1, fully exposed) its
                    # ~10us serial latency is on the critical path, so those two
                    # run the chain in two independent 512-pixel halves.
                    split = (t == 0 and d == 0) or (t == T - 1 and d == 1)
                    halves = (0, 1) if split else (slice(0, 2),)
                    for hsel in halves:
                        cs = slice(0, HWSZ) if isinstance(hsel, slice) else slice(512 * hsel, 512 * hsel + 512)
                        nrow = H if isinstance(hsel, slice) else 16
                        r0 = 0 if isinstance(hsel, slice) else 16 * hsel
                        nc.scalar.activation(sif[:, hsel], ps0[:, hsel], AF.Sigmoid, bias=bias[:, d, 0:1])
                        nc.scalar.activation(sgo[0:64, hsel], ps1[0:64, hsel], AF.Tanh, bias=bias[0:64, d, 1:2])
                        nc.scalar.activation(sgo[64:128, hsel], ps1[64:128, hsel], AF.Sigmoid, bias=bias[64:128, d, 1:2])
                        # i*g on partitions 0-63, then ship it up to 64-127 where f/o live
                        nc.vector.tensor_mul(tmp[:, cs], sif[0:64, hsel], sgo[0:64, hsel])
                        nc.sync.dma_start(tup[64:128, cs], tmp[:, cs])
                        if t > 0:
                            nc.vector.tensor_mul(c2[d][64:128, cs], c2[d][64:128, cs], sif[64:128, hsel])
                            nc.vector.tensor_add(c2[d][64:128, cs], c2[d][64:128, cs], tup[64:128, cs])
                        else:
                            nc.vector.tensor_copy(c2[d][64:128, cs], tup[64:128, cs])
                        nc.scalar.activation(tup[64:128, cs], c2[d][64:128, cs], AF.Tanh)
                        # h = o * tanh(c), entirely on partitions 64-127
                        nc.vector.tensor_mul(h2c[64:128, cs], sgo[64:128, hsel], tup[64:128, cs])
                        nc.scalar.dma_start(out_d[tsrc, d, :, r0 : r0 + nrow], h2c[64:128, cs])
                        if t < T - 1:
                            ofl = (
                                sgo[64:128, hsel].rearrange("p a b -> p (a b)")
                                if isinstance(hsel, slice)
                                else sgo[64:128, hsel]
                            )
                            o3 = ofl.rearrange("p (a b) -> p a b", a=nrow)
                            th3 = tup[64:128, cs].rearrange("p (a b) -> p a b", a=nrow)
                            # shifted upper state copies written directly by lane-aligned DVE
                            nc.vector.tensor_mul(hAB[d][64:128, 0, 2 + r0 : 2 + r0 + nrow, 3:35], o3, th3)
                            nc.vector.tensor_mul(hAB[d][64:128, 1, 3 + r0 : 3 + r0 + nrow, 2:34], o3, th3)
                            # lower copies: ship h down to partitions 0-63, broadcast-write both
                            nc.sync.dma_start(hl[:, cs], h2c[64:128, cs])
                            hl4 = hl[:, cs].rearrange("p (a b) -> p a b", a=nrow).unsqueeze(1).to_broadcast([HID, 2, nrow, W])
                            nc.vector.tensor_copy(hAB[d][0:64, :, 3 + r0 : 3 + r0 + nrow, 3:35], hl4)
    nc.compile()
    return nc


_CACHE = {}


def get_nc():
    if "nc" not in _CACHE:
        _CACHE["nc"] = build_nc()
    return _CACHE["nc"]


def make_in_maps(inputs):
    wih0, wih1 = pack_wih(inputs["w_ih_f"], inputs["w_ih_b"])
    shared = {
        "whh": pack_whh(inputs["w_hh_f"], inputs["w_hh_b"]),
        "wih0": wih0,
        "wih1": wih1,
        "bias": pack_bias(
            inputs["b_ih_f"], inputs["b_hh_f"], inputs["b_ih_b"], inputs["b_hh_b"]
        ),
    }
    x = np.asarray(inputs["x"], dtype=np.float32)
    maps = []
    for b in range(NCORES):
        xa, xb4 = pack_xcol(x[b])
        maps.append(dict(shared, xcola=xa, xcolb=xb4))
    return maps


def assemble(results):
    final = np.empty((NCORES, T, 2 * HID, H, W), np.float32)
    for b in range(NCORES):
        ob = results[b]["out"]  # [T, 2, HID, H, W]
        final[b, :, 0:HID] = ob[:, 0]
        final[b, :, HID:] = ob[:, 1]
    return final


def run_on_device(inputs, **kwargs):
    from concourse.bass_utils import run_bass_kernel_spmd

    nc = get_nc()
    in_maps = make_in_maps(inputs)
    res = run_bass_kernel_spmd(nc, in_maps, core_ids=list(range(NCORES)), **kwargs)
    return assemble(res.results), res


def kernel(**inputs):
    out, _ = run_on_device(inputs)
    return out


# revision 3
# speedup vs baseline: 1.0024x; 1.0024x over previous
"""Bidirectional ConvLSTM encoder kernel for Trainium2 (Bass/Tile).

Problem: B=8, T=16, C=3, H=W=32, HID=64, 7x7 convs, bidirectional.
Sharding: data-parallel over batch; core b handles batch element b, running
both the forward and backward recurrences (2 independent recurrences that
ping-pong on the PE so gate/elementwise latency of one hides under the
other's matmuls).

Conv formulation: hidden 7x7 conv (64->256ch) as a sum of shifted matmuls
over a zero-padded [64, 38, 38] state image, taps packed in pairs onto the
128-deep contraction dim via two shifted state copies (see pack_whh).
25 matmuls replace 49. The 3x7x7 input conv is im2col'd (K=147, padded to
160) and accumulated into the same PSUM banks.

v3 over baseline:
- K=32 input-conv tile: weights+xcol replicated at partition offsets
  0/32/64/96 so the four (mg,nh) matmuls run CONCURRENTLY in distinct
  32-row groups of the PE via tile_position (4 MMs -> ~1 MM span).
- Singleton hidden tap (6,6) (K=64): weights duplicated on partitions
  64:128; the nh=1 matmul reads the col-shifted upper state copy at column
  base 5 (same values) and runs in row groups 2-3 concurrently with nh=0
  in row groups 0-1 via tile_position (2 MMs -> ~1 MM span per mg).
- Startup: only the t=0 slice of the im2col input is DMA'd before compute
  starts (full tensor was ~4MB, ~12us of PE idle); weights and the
  remaining timesteps stream in behind it.
"""

import numpy as np

HID = 64
T = 16
CIN = 3
H = 32
W = 32
HWSZ = H * W
PW = 38  # padded image width (32 + 2*3)
PAD = 3
KS = 7
NCORES = 8
KIN = CIN * KS * KS  # 147
KIN_PAD = 160  # 128 + 32
# xcol timestep slots in PE consumption order: t=0 fwd, t=15 bwd, t=1 fwd, ...
PERM = [0, 15, 1, 14, 2, 13, 3, 12, 4, 11, 5, 10, 6, 9, 7, 8]
SLOT = {t: PERM.index(t) for t in range(T)}
# whh pair slots in PE consumption order: S pair (index 24) first, then 0..23
PPERM = [24] + list(range(24))
PSLOT = {p: PPERM.index(p) for p in range(25)}

# Hidden-conv tap pairs: (kind, kh, kw).
#  "A": taps (kh, kw) + (kh+1, kw) via the row-shifted upper copy.
#  "B": taps (6, kw) + (6, kw+1) via the col-shifted upper copy.
#  "S": singleton tap (6, 6), K=64, row-tiled 2x (nh=0 lo / nh=1 hi).
PAIRS = (
    [("A", kh0, kw) for kw in range(KS) for kh0 in (0, 2, 4)]
    + [("B", 6, kw0) for kw0 in (0, 2, 4)]
    + [("S", 6, 6)]
)
NPAIR = len(PAIRS)  # 25


def pack_whh(w_hh_f: np.ndarray, w_hh_b: np.ndarray) -> np.ndarray:
    """Pack hidden weights into lhsT tiles: [128(k), 2(dir), 25(pair), 2(mg), 128(m)].

    lhsT[k, d, p, mg, m] so that matmul(lhsT.T @ rhs) with rhs rows
    (k<64: tap_lo channel k, k>=64: tap_hi channel k-64) accumulates the conv.
    The S pair carries the same tap on both halves (row-tiled concurrent MMs).
    """
    out = np.zeros((2, NPAIR, 2, 128, 128), np.float32)  # d, p, mg, k, m
    for d, wsrc in enumerate([w_hh_f, w_hh_b]):
        wsrc = np.asarray(wsrc, dtype=np.float32)  # [256, 64, 7, 7]
        for p, (kind, r, c) in enumerate(PAIRS):
            if kind == "A":
                lo, hi = (r, c), (r + 1, c)
            elif kind == "B":
                lo, hi = (r, c), (r, c + 1)
            else:
                lo, hi = (r, c), (r, c)  # duplicated for the row-tiled variant
            for mg in range(2):
                wm = wsrc[mg * 128 : (mg + 1) * 128]  # [128, 64, 7, 7]
                out[d, p, mg, 0:64, :] = wm[:, :, lo[0], lo[1]].T
                out[d, p, mg, 64:128, :] = wm[:, :, hi[0], hi[1]].T
    out = out[:, PPERM]  # consumption-ordered pair slots
    return np.ascontiguousarray(out.transpose(3, 0, 1, 2, 4).astype(np.float16))  # [k, d, p, mg, m]


def pack_wih(w_ih_f: np.ndarray, w_ih_b: np.ndarray) -> tuple[np.ndarray, np.ndarray]:
    """Input weights (im2col).

    wih0: [128(k), 2(dir), 2(mg), 128(m)] — first K tile.
    wih1: [128(k), 2(dir), 2(mg), 128(m)] — rows 128:160 replicated at
          partition offsets 0/32/64/96 for 4x row-tiled matmuls.
    """
    w0 = np.zeros((128, 2, 2, 128), np.float32)
    w1 = np.zeros((128, 2, 2, 128), np.float32)
    for d, wsrc in enumerate([w_ih_f, w_ih_b]):
        wk = np.asarray(wsrc, dtype=np.float32).reshape(256, KIN)  # (cin,kh,kw) C-order
        for mg in range(2):
            w0[:, d, mg, :] = wk[mg * 128 : (mg + 1) * 128, 0:128].T
            tail = wk[mg * 128 : (mg + 1) * 128, 128:KIN].T  # [19, 128]
            for i in range(4):
                w1[32 * i : 32 * i + 19, d, mg, :] = tail
    return (
        np.ascontiguousarray(w0.astype(np.float16)),
        np.ascontiguousarray(w1.astype(np.float16)),
    )


def pack_bias(b_ih_f, b_hh_f, b_ih_b, b_hh_b) -> np.ndarray:
    """[128(k), 2(dir), 2(mg)]: per-gate-channel bias."""
    out = np.zeros((128, 2, 2), np.float32)
    for d, (bi, bh) in enumerate([(b_ih_f, b_hh_f), (b_ih_b, b_hh_b)]):
        s = np.asarray(bi, dtype=np.float32) + np.asarray(bh, dtype=np.float32)  # [256]
        out[:, d, 0] = s[0:128]
        out[:, d, 1] = s[128:256]
    return np.ascontiguousarray(out)


def pack_xcol(xb: np.ndarray) -> tuple[np.ndarray, np.ndarray]:
    """im2col one batch element [T,3,32,32].

    xcola: [128(k), T, 2, 512] — first K tile.
    xcolb: [128(k), T, 2, 512] — rows 128:147 replicated at partition
           offsets 0/32/64/96 (matching pack_wih's wih1).
    """
    xb = np.asarray(xb, dtype=np.float32)
    xpad = np.pad(xb, ((0, 0), (0, 0), (PAD, PAD), (PAD, PAD)))
    win = np.lib.stride_tricks.sliding_window_view(xpad, (KS, KS), axis=(2, 3))
    # win: [T, 3, 32, 32, 7, 7] -> [(cin, kh, kw), T, hw]
    xcol = win.transpose(1, 4, 5, 0, 2, 3).reshape(KIN, T, HWSZ).astype(np.float16)
    xcol = xcol[:, PERM]  # consumption-ordered slots
    xa = np.ascontiguousarray(xcol[0:128].reshape(128, T, 2, 512))
    tail = xcol[128:KIN].reshape(19, T, 2, 512)
    xb4 = np.zeros((128, T, 2, 512), np.float16)
    for i in range(4):
        xb4[32 * i : 32 * i + 19] = tail
    return xa, xb4


def build_nc():
    import concourse.mybir as mybir
    from concourse import bacc
    from concourse.tile import TileContext

    F32 = mybir.dt.float32
    F16 = mybir.dt.float16
    AF = mybir.ActivationFunctionType

    nc = bacc.Bacc()
    xcola_d = nc.declare_dram_parameter("xcola", [128, T, 2, 512], F16, isOutput=False)
    xcolb_d = nc.declare_dram_parameter("xcolb", [128, T, 2, 512], F16, isOutput=False)
    whh_d = nc.declare_dram_parameter("whh", [128, 2, NPAIR, 2, 128], F16, isOutput=False)
    wih0_d = nc.declare_dram_parameter("wih0", [128, 2, 2, 128], F16, isOutput=False)
    wih1_d = nc.declare_dram_parameter("wih1", [128, 2, 2, 128], F16, isOutput=False)
    bias_d = nc.declare_dram_parameter("bias", [128, 2, 2], F32, isOutput=False)
    out_d = nc.declare_dram_parameter("out", [T, 2, HID, H, W], F32, isOutput=True)

    with TileContext(nc) as tc:
        with (
            tc.tile_pool(name="wpool", bufs=1) as wpool,
            tc.tile_pool(name="state", bufs=1) as spool,
            tc.tile_pool(name="xin", bufs=1) as xpool,
            tc.tile_pool(name="work", bufs=1) as wkpool,
            tc.tile_pool(name="psum", bufs=1, space="PSUM") as pspool,
        ):
            whh = wpool.tile([128, 2, NPAIR, 2, 128], F16)
            wih0 = wpool.tile([128, 2, 2, 128], F16)
            wih1 = wpool.tile([128, 2, 2, 128], F16)
            bias = wpool.tile([128, 2, 2], F32)
            xaF = xpool.tile([128, T, 2, 512], F16)
            xbF = xpool.tile([128, T, 2, 512], F16)

            # Startup choreography. xcol slots are in consumption order
            # (PERM), so chunks are contiguous. sync and scalar are
            # independent FIFO HW-DGE rings: sync carries only the small
            # critical head (so the per-step gate DMAs behind it are never
            # delayed); scalar carries six big bulk transfers whose
            # deadlines are far out. Activations on the scalar engine only
            # have to get past 6 quick DMA issues.
            nc.sync.dma_start(wih0[:], wih0_d[:])
            nc.sync.dma_start(wih1[:], wih1_d[:])
            nc.sync.dma_start(bias[:], bias_d[:])
            nc.sync.dma_start(xaF[:, 0:1, 0], xcola_d[:, 0:1, 0])
            nc.sync.dma_start(xaF[:, 0:1, 1], xcola_d[:, 0:1, 1])
            nc.sync.dma_start(xbF[:, 0:1], xcolb_d[:, 0:1])
            nc.sync.dma_start(xaF[:, 1:2], xcola_d[:, 1:2])
            nc.sync.dma_start(xbF[:, 1:2], xcolb_d[:, 1:2])
            nc.scalar.dma_start(whh[:, 0, 0:9], whh_d[:, 0, 0:9])
            nc.scalar.dma_start(xaF[:, 2:4], xcola_d[:, 2:4])
            nc.scalar.dma_start(xbF[:, 2:4], xcolb_d[:, 2:4])
            nc.scalar.dma_start(whh[:, 0, 9:25], whh_d[:, 0, 9:25])
            # warm the scalar engine's sigmoid/tanh tables while the input
            # DMAs stream, so the first gate activation doesn't pay the
            # ~2.6us ACT_TABLE_LOAD on the critical path
            warm = wkpool.tile([1, 2], F32, tag="warm")
            nc.gpsimd.memset(warm[:], 0.0)
            nc.scalar.activation(warm[:], warm[:], AF.Sigmoid)
            nc.scalar.activation(warm[:], warm[:], AF.Tanh)

            hAB = [spool.tile([128, 2, PW, PW], F16, tag=f"hAB{d}", name=f"hAB{d}") for d in range(2)]
            # cell state lives on partitions 64-127, where the f and o gates land
            c2 = [spool.tile([128, HWSZ], F32, tag=f"c{d}", name=f"c{d}") for d in range(2)]
            for tl in hAB:
                nc.vector.memset(tl[:], 0.0)

            for t in range(T):
                if t == 1:
                    # bulk transfers all on the scalar ring: the sync ring
                    # must stay clear for the latency-critical gate DMAs
                    # (the scheduler hoists these issues as soon as deps
                    # allow, cutting in line ahead of not-yet-ready gate
                    # DMAs on the same ring)
                    nc.scalar.dma_start(whh[:, 1, 0:9], whh_d[:, 1, 0:9])
                    nc.scalar.dma_start(whh[:, 1, 9:25], whh_d[:, 1, 9:25])
                    nc.scalar.dma_start(xaF[:, 4:10], xcola_d[:, 4:10])
                    nc.scalar.dma_start(xbF[:, 4:10], xcolb_d[:, 4:10])
                elif t == 2:
                    nc.scalar.dma_start(xaF[:, 10:16], xcola_d[:, 10:16])
                    nc.scalar.dma_start(xbF[:, 10:16], xcolb_d[:, 10:16])
                for d in range(2):
                    tsrc = t if d == 0 else T - 1 - t

                    ps0 = pspool.tile([128, 2, 512], F32, tag=f"ps{d}0")
                    ps1 = pspool.tile([128, 2, 512], F32, tag=f"ps{d}1")
                    pst = [ps0, ps1]

                    # K=128 input tile: start of each (mg, nh) group
                    for mg in range(2):
                        for nh in range(2):
                            nc.tensor.matmul(
                                pst[mg][:, nh], wih0[:, d, mg], xaF[:, SLOT[tsrc], nh],
                                start=True, stop=False,
                            )
                    # K=32 input tail: all four (mg, nh) matmuls adjacent so
                    # they run concurrently in distinct 32-row groups
                    for mg in range(2):
                        for nh in range(2):
                            i = 2 * mg + nh
                            nc.tensor.matmul(
                                pst[mg][:, nh],
                                wih1[32 * i : 32 * i + 32, d, mg],
                                xbF[32 * i : 32 * i + 32, SLOT[tsrc], nh],
                                start=False, stop=(t == 0),
                                tile_position=(32 * i, 0),
                            )
                    if t > 0:
                        # S singleton (K=64) row-tiled pairs, grouped right
                        # after the K=32 matmuls so the PE pays the tile-
                        # mode-switch stall once per step instead of per
                        # group. nh=0 on rows 0:64 (plain state in B-copy
                        # lower), nh=1 on rows 64:128 (col-shifted B-upper
                        # at column base 5 — same values).
                        ps_idx = PSLOT[NPAIR - 1]
                        for mg in range(2):
                            nc.tensor.matmul(
                                pst[mg][:, 0],
                                whh[0:64, d, ps_idx, mg],
                                hAB[d][0:64, 1, 6:22, 6:38],
                                start=False, stop=False,
                                tile_position=(0, 0),
                            )
                            nc.tensor.matmul(
                                pst[mg][:, 1],
                                whh[64:128, d, ps_idx, mg],
                                hAB[d][64:128, 1, 22:38, 5:37],
                                start=False, stop=False,
                                tile_position=(64, 0),
                            )
                        for mg in range(2):
                            for p, (kind, r, c) in enumerate(PAIRS[:-1]):
                                if kind == "A":
                                    rhf = lambda nh, r=r, c=c: hAB[d][:, 0, r + 16 * nh : r + 16 * nh + 16, c : c + 32]
                                else:
                                    rhf = lambda nh, c=c: hAB[d][:, 1, 6 + 16 * nh : 6 + 16 * nh + 16, c : c + 32]
                                last = p == NPAIR - 2
                                for nh in range(2):
                                    nc.tensor.matmul(
                                        pst[mg][:, nh],
                                        whh[:, d, PSLOT[p], mg],
                                        rhf(nh),
                                        start=False, stop=last,
                                    )

                    sif = wkpool.tile([128, 2, 512], F32, tag=f"sif{d}")
                    sgo = wkpool.tile([128, 2, 512], F32, tag=f"sgo{d}")
                    tmp = wkpool.tile([HID, HWSZ], F32, tag=f"tmp{d}")
                    tup = wkpool.tile([128, HWSZ], F32, tag=f"tup{d}")
                    h2c = wkpool.tile([128, HWSZ], F32, tag=f"h2c{d}{t % 3}")
                    hl = wkpool.tile([HID, HWSZ], F32, tag=f"hl{d}")

                    # gates: i,f = sigmoid(mg0); g = tanh(mg1 lo); o = sigmoid(mg1 hi).
                    # Normally the whole chain hides under the other direction's
                    # matmul stream; at the kernel head (t=0 d=0, stalls t=1's
                    # hidden matmuls) and tail (last step d=1, fully exposed) its
                    # ~10us serial latency is on the critical path, so those two
                    # run the chain in two independent 512-pixel halves.
                    split = (t == 0 and d == 0) or (t == T - 1 and d == 1)
                    halves = (0, 1) if split else (slice(0, 2),)
                    for hsel in halves:
                        cs = slice(0, HWSZ) if isinstance(hsel, slice) else slice(512 * hsel, 512 * hsel + 512)
                        nrow = H if isinstance(hsel, slice) else 16
                        r0 = 0 if isinstance(hsel, slice) else 16 * hsel
                        nc.scalar.activation(sif[:, hsel], ps0[:, hsel], AF.Sigmoid, bias=bias[:, d, 0:1])
                        nc.scalar.activation(sgo[0:64, hsel], ps1[0:64, hsel], AF.Tanh, bias=bias[0:64, d, 1:2])
                        nc.scalar.activation(sgo[64:128, hsel], ps1[64:128, hsel], AF.Sigmoid, bias=bias[64:128, d, 1:2])
                        # i*g on partitions 0-63, then ship it up to 64-127 where f/o live
                        nc.vector.tensor_mul(tmp[:, cs], sif[0:64, hsel], sgo[0:64, hsel])
                        nc.sync.dma_start(tup[64:128, cs], tmp[:, cs])
                        if t > 0:
                            nc.vector.tensor_mul(c2[d][64:128, cs], c2[d][64:128, cs], sif[64:128, hsel])
                            nc.vector.tensor_add(c2[d][64:128, cs], c2[d][64:128, cs], tup[64:128, cs])
                        else:
                            nc.vector.tensor_copy(c2[d][64:128, cs], tup[64:128, cs])
                        nc.scalar.activation(tup[64:128, cs], c2[d][64:128, cs], AF.Tanh)
                        # h = o * tanh(c), entirely on partitions 64-127
                        nc.vector.tensor_mul(h2c[64:128, cs], sgo[64:128, hsel], tup[64:128, cs])
                        nc.scalar.dma_start(out_d[tsrc, d, :, r0 : r0 + nrow], h2c[64:128, cs])
                        if t < T - 1:
                            ofl = (
                                sgo[64:128, hsel].rearrange("p a b -> p (a b)")
                                if isinstance(hsel, slice)
                                else sgo[64:128, hsel]
                            )
                            o3 = ofl.rearrange("p (a b) -> p a b", a=nrow)
                            th3 = tup[64:128, cs].rearrange("p (a b) -> p a b", a=nrow)
                            # shifted upper state copies written directly by lane-aligned DVE
                            nc.vector.tensor_mul(hAB[d][64:128, 0, 2 + r0 : 2 + r0 + nrow, 3:35], o3, th3)
                            nc.vector.tensor_mul(hAB[d][64:128, 1, 3 + r0 : 3 + r0 + nrow, 2:34], o3, th3)
                            # lower copies: ship h down to partitions 0-63, broadcast-write both
                            nc.sync.dma_start(hl[:, cs], h2c[64:128, cs])
                            hl4 = hl[:, cs].rearrange("p (a b) -> p a b", a=nrow).unsqueeze(1).to_broadcast([HID, 2, nrow, W])
                            nc.vector.tensor_copy(hAB[d][0:64, :, 3 + r0 : 3 + r0 + nrow, 3:35], hl4)
    nc.compile()
    return nc


_CACHE = {}


def get_nc():
    if "nc" not in _CACHE:
        _CACHE["nc"] = build_nc()
    return _CACHE["nc"]


def make_in_maps(inputs):
    wih0, wih1 = pack_wih(inputs["w_ih_f"], inputs["w_ih_b"])
    shared = {
        "whh": pack_whh(inputs["w_hh_f"], inputs["w_hh_b"]),
        "wih0": wih0,
        "wih1": wih1,
        "bias": pack_bias(
            inputs["b_ih_f"], inputs["b_hh_f"], inputs["b_ih_b"], inputs["b_hh_b"]
        ),
    }
    x = np.asarray(inputs["x"], dtype=np.float32)
    maps = []
    for b in range(NCORES):
        xa, xb4 = pack_xcol(x[b])
        maps.append(dict(shared, xcola=xa, xcolb=xb4))
    return maps


def assemble(results):
    final = np.empty((NCORES, T, 2 * HID, H, W), np.float32)
    for b in range(NCORES):
        ob = results[b]["out"]  # [T, 2, HID, H, W]
        final[b, :, 0:HID] = ob[:, 0]
        final[b, :, HID:] = ob[:, 1]
    return final


def run_on_device(inputs, **kwargs):
    from concourse.bass_utils import run_bass_kernel_spmd

    nc = get_nc()
    in_maps = make_in_maps(inputs)
    res = run_bass_kernel_spmd(nc, in_maps, core_ids=list(range(NCORES)), **kwargs)
    return assemble(res.results), res


def kernel(**inputs):
    out, _ = run_on_device(inputs)
    return out
